# revision 3
# baseline (speedup 1.0000x reference)
"""Trainium2 Bass kernel for DeepSelfAttention (N=8192, D=1024) on 8 NeuronCores.

Strategy (row-parallel attention):
  - Shard the N=8192 rows of x across 8 cores (1024 rows each); replicate weights.
  - All operand transposes (x and the six d x d weights) are done by the DMA
    XBAR (fp32 natural load -> ScalarE fp16 cast -> 8 dma_start_transpose per
    matrix), keeping the TensorEngine free for matmuls.
  - Each core computes Q/K/V projections for its row shard feature-major;
    K^T and V are shipped per key-half: (K^T h0, V h0) -> AllGather0,
    (K^T h1, V h1) -> AllGather1, so the first collective starts as early
    as possible; Q projection + MLP weight transposes fill its latency.
  - Flash-style one-pass attention: scores^T tiles [k=128, q=512] accumulate
    over feature tiles in PSUM, exp on ScalarE (scale=1/32 fused; scores are
    provably in [-3, 3] so no max-subtraction), A@V per (block, dt) with
    free-dim 512 into a rotating set of 4 PSUM banks, flushed to an SBUF
    fp32 accumulator; softmax denominator via a ones-vector matmul.
  - The V bias is folded into the post-softmax normalize (softmax rows sum
    to 1), where it is a per-partition bias.
  - 3-layer MLP + final projection, feature-major.
All matmul operands are fp16 (full PE rate on TRN2) with fp32 PSUM
accumulation; end-to-end max rel err vs the fp32 reference is ~4e-4.
"""

import os

import numpy as np

import concourse.mybir as mybir
import concourse.tile as tile
from concourse import bacc
from concourse import bass_utils

P = 128
D = 1024
N = 8192
NCORES = 8
NS = N // NCORES          # 1024 rows per core
DT = D // P               # 8 feature tiles
KB = 8                    # k blocks (one per source core)
KTB = NS // P             # 8 k tiles per block
KTH = KTB // 2            # 4 k tiles per chunk-block
CH = NS // 2              # 512 keys per chunk
KSZ = D * CH              # K-chunk elements in the flat collective buffer
VSZ = CH * D
F16 = mybir.dt.float16
F32 = mybir.dt.float32
AF = mybir.ActivationFunctionType
ALU = mybir.AluOpType

SCALE = 1.0 / np.sqrt(np.float32(D)).astype(np.float32)  # 0.03125

_CACHE = {}


def _transpose_xbar(nc, st32, st16, src_ap, dst_tile):
    """src_ap: DRAM fp32 [R, C] -> dst_tile: SBUF fp16 [P, C//P, R] = src.T,
    via natural load + ScalarE fp16 cast + DMA XBAR transposes, staged in
    row-halves to bound SBUF usage."""
    R, C = src_ap.shape
    tb = R // P // 2
    for rh in range(2):
        nat = st32.tile([P, tb, C], F32, tag="st32")
        nc.sync.dma_start(
            nat[:],
            src_ap[rh * (R // 2):(rh + 1) * (R // 2), :].rearrange(
                "(t p) c -> p t c", p=P))
        nath = st16.tile([P, tb, C], F16, tag="st16")
        nc.scalar.copy(nath[:], nat[:])
        for t in range(tb):
            nc.sync.dma_start_transpose(
                dst_tile[:, :, (rh * tb + t) * P:(rh * tb + t + 1) * P],
                nath[:, t, :])


def _build():
    nc = bacc.Bacc("TRN2", target_bir_lowering=False, debug=False,
                   num_devices=NCORES)
    xs = nc.dram_tensor("xs", [NS, D], F32, kind="ExternalInput").ap()
    W = {}
    for w in ("wq", "wk", "wv", "w1", "w2", "w3"):
        W[w] = nc.dram_tensor(w, [D, D], F32, kind="ExternalInput").ap()
    B = {}
    for b in ("bq", "bk", "bv", "b1", "b2", "b3"):
        B[b] = nc.dram_tensor(b, [D], F32, kind="ExternalInput").ap()
    fw = nc.dram_tensor("fw", [D], F32, kind="ExternalInput").ap()
    out = nc.dram_tensor("out", [1, NS], F32, kind="ExternalOutput").ap()
    debug = bool(os.environ.get("K_DEBUG"))
    dbg = {}
    if debug:
        for nm, shp, dt_ in (("dq", [D, NS], F16), ("drs", [1, NS], F32),
                             ("datt", [D, NS], F16), ("dy1", [D, NS], F16)):
            dbg[nm] = nc.dram_tensor(nm, shp, dt_, kind="ExternalOutput").ap()

    with tile.TileContext(nc) as tc:
        with (
            tc.tile_pool(name="persist", bufs=1) as pers,
            tc.tile_pool(name="dram", bufs=1, space="DRAM") as dram,
        ):
            # ---- persistent SBUF tiles ----
            qt = pers.tile([P, DT, NS], F16, tag="qt")          # Q^T
            wT = {w: pers.tile([P, DT, D], F16, tag=f"{w}T", name=f"{w}T")
                  for w in ("w1", "w2", "w3")}
            bsb = {b: pers.tile([P, DT], F32, tag=f"{b}sb", name=f"{b}sb")
                   for b in B}
            fwh = pers.tile([P, DT], F16, tag="fwh")
            ones_h = pers.tile([P, 1], F16, tag="ones")
            ones_row = pers.tile([1, P], F32, tag="ones_row")
            rs = pers.tile([1, NS], F32, tag="rs")              # softmax denom

            # ---- DRAM scratch: flat (K-chunk | V-chunk) collective buffers
            kv_d = [dram.tile([KSZ + VSZ], F16, name=f"kv_d{c}")
                    for c in range(2)]
            kvag = [dram.tile([NCORES * (KSZ + VSZ)], F16, name=f"kvag{c}",
                              addr_space="Shared")
                    for c in range(2)]

            # ---- constants ----
            for b in B:
                nc.sync.dma_start(bsb[b][:], B[b].rearrange("(t p) -> p t", p=P))
            fwf = pers.tile([P, DT], F32, tag="fwf")
            nc.sync.dma_start(fwf[:], fw.rearrange("(t p) -> p t", p=P))
            nc.vector.tensor_copy(fwh[:], fwf[:])
            nc.gpsimd.memset(ones_h[:], 1.0)
            nc.gpsimd.memset(ones_row[:], 1.0)

            # ---- early pool: dies after projections ----
            early = tc.alloc_tile_pool(name="early", bufs=1)
            xsT = early.tile([P, DT, NS], F16, tag="xsT")
            for w in ("wq", "wk", "wv"):
                wT[w] = early.tile([P, DT, D], F16, tag=f"{w}T", name=f"{w}T")
            kts = early.tile([P, DT, NS], F16, tag="kts")       # K^T shard
            vs = early.tile([P, KTB, D], F16, tag="vs")         # V shard

            with (
                tc.tile_pool(name="st32", bufs=2) as st32,
                tc.tile_pool(name="st16", bufs=1) as st16,
                tc.tile_pool(name="ppj", bufs=4, space="PSUM") as ppj,
            ):
                # XBAR transposes: x, then K/V weights (gate AllGather0)
                _transpose_xbar(nc, st32, st16, xs, xsT)
                for w in ("wk", "wv"):
                    _transpose_xbar(nc, st32, st16, W[w], wT[w])

                # per key-half: K^T, V, ship, AllGather
                for h in range(2):
                    # K^T[:, half] = Wk @ xs^T + bk
                    for dt in range(DT):
                        ps = ppj.tile([P, 512], F32, tag="ppj")
                        for et in range(DT):
                            nc.tensor.matmul(
                                ps[:],
                                wT["wk"][:, et, dt * P:(dt + 1) * P],
                                xsT[:, et, h * 512:(h + 1) * 512],
                                start=(et == 0), stop=(et == DT - 1))
                        nc.scalar.activation(
                            kts[:, dt, h * 512:(h + 1) * 512], ps[:],
                            AF.Identity, bias=bsb["bk"][:, dt:dt + 1])
                    nc.sync.dma_start(
                        kv_d[h][0:KSZ].rearrange("(t p k) -> p t k", p=P, k=CH),
                        kts[:, :, h * CH:(h + 1) * CH])
                    # V[half] = xs @ Wv.T (bias folded into the normalize)
                    for kt in range(h * KTH, (h + 1) * KTH):
                        for dh in range(2):
                            ps = ppj.tile([P, 512], F32, tag="ppj")
                            for et in range(DT):
                                nc.tensor.matmul(
                                    ps[:],
                                    xsT[:, et, kt * P:(kt + 1) * P],
                                    wT["wv"][:, et, dh * 512:(dh + 1) * 512],
                                    start=(et == 0), stop=(et == DT - 1))
                            nc.scalar.copy(
                                vs[:, kt, dh * 512:(dh + 1) * 512], ps[:])
                    nc.sync.dma_start(
                        kv_d[h][KSZ:].rearrange("(t p d) -> p t d", p=P, d=D),
                        vs[:, h * KTH:(h + 1) * KTH, :])
                    nc.gpsimd.collective_compute(
                        "AllGather", ALU.bypass,
                        replica_groups=[list(range(NCORES))],
                        ins=[kv_d[h].opt()], outs=[kvag[h].opt()])

                # work that fills the collective latency: Q^T projection
                # + MLP weight transposes
                _transpose_xbar(nc, st32, st16, W["wq"], wT["wq"])
                for dt in range(DT):
                    for h in range(2):
                        ps = ppj.tile([P, 512], F32, tag="ppj")
                        for et in range(DT):
                            nc.tensor.matmul(
                                ps[:],
                                wT["wq"][:, et, dt * P:(dt + 1) * P],
                                xsT[:, et, h * 512:(h + 1) * 512],
                                start=(et == 0), stop=(et == DT - 1))
                        nc.scalar.activation(
                            qt[:, dt, h * 512:(h + 1) * 512], ps[:],
                            AF.Identity, bias=bsb["bq"][:, dt:dt + 1])
                for w in ("w1", "w2", "w3"):
                    _transpose_xbar(nc, st32, st16, W[w], wT[w])

            early.release()

            if debug:
                nc.sync.dma_start(dbg["dq"].rearrange("(t p) k -> p t k", p=P),
                                  qt[:])

            # ---- attention over 2 chunks x 8 blocks ----
            pacc = tc.alloc_tile_pool(name="pacc", bufs=1)
            attacc = pacc.tile([P, DT, NS], F32, tag="attacc")
            with (
                tc.tile_pool(name="kv", bufs=3) as kv,
                tc.tile_pool(name="ex", bufs=8) as exp_pool,
                tc.tile_pool(name="psc", bufs=2, space="PSUM") as psc,
                tc.tile_pool(name="pat", bufs=4, space="PSUM") as pat,
                tc.tile_pool(name="prs", bufs=2, space="PSUM") as prs,
            ):
                for ch in range(2):
                    base = kvag[ch]
                    for kb in range(KB):
                        off = kb * (KSZ + VSZ)
                        ktb = kv.tile([P, DT, CH], F16, tag="ktb")
                        vb = kv.tile([P, KTH, D], F16, tag="vb")
                        nc.sync.dma_start(
                            ktb[:],
                            base[off:off + KSZ].rearrange(
                                "(t p k) -> p t k", p=P, k=CH))
                        nc.sync.dma_start(
                            vb[:],
                            base[off + KSZ:off + KSZ + VSZ].rearrange(
                                "(t p d) -> p t d", p=P, d=D))
                        first_blk = ch == 0 and kb == 0
                        for qp in range(2):
                            qpsl = slice(qp * 512, (qp + 1) * 512)
                            rs_ps = prs.tile([1, 512], F32, tag="prs")
                            exs = []
                            for kt in range(KTH):
                                sc = psc.tile([P, 512], F32, tag="psc")
                                for dt in range(DT):
                                    nc.tensor.matmul(
                                        sc[:],
                                        ktb[:, dt, kt * P:(kt + 1) * P],
                                        qt[:, dt, qpsl],
                                        start=(dt == 0), stop=(dt == DT - 1))
                                ex = exp_pool.tile([P, 512], F16, tag="ex",
                                                   name=f"ex{kt}")
                                nc.scalar.activation(ex[:], sc[:], AF.Exp,
                                                     scale=float(SCALE))
                                nc.tensor.matmul(rs_ps[:], ones_h[:], ex[:],
                                                 start=(kt == 0),
                                                 stop=(kt == KTH - 1),
                                                 skip_group_check=True)
                                exs.append(ex)
                            if first_blk:
                                nc.vector.tensor_copy(rs[0:1, qpsl], rs_ps[:])
                            else:
                                nc.vector.tensor_tensor(
                                    rs[0:1, qpsl], rs_ps[:], rs[0:1, qpsl],
                                    ALU.add)
                            # A@V: per dt, accumulate the 4 kt matmuls in one
                            # PSUM bank (free dim 512), 4 banks rotating
                            for dt in range(DT):
                                att_ps = pat.tile([P, 512], F32, tag="pat")
                                for kt in range(KTH):
                                    nc.tensor.matmul(
                                        att_ps[:],
                                        vb[:, kt, dt * P:(dt + 1) * P],
                                        exs[kt][:],
                                        start=(kt == 0),
                                        stop=(kt == KTH - 1))
                                dsl = (slice(None), dt, qpsl)
                                if first_blk:
                                    nc.vector.tensor_copy(attacc[dsl],
                                                          att_ps[:])
                                else:
                                    nc.vector.tensor_tensor(
                                        attacc[dsl], att_ps[:],
                                        attacc[dsl], ALU.add)

            # ---- normalize + MLP + final ----
            with (
                tc.tile_pool(name="acts", bufs=2) as acts,
                tc.tile_pool(name="pml", bufs=4, space="PSUM") as pml,
            ):
                recip = acts.tile([1, NS], F32, tag="recip")
                out_sb = acts.tile([1, NS], F32, tag="out_sb")
                nc.vector.reciprocal(recip[:], rs[:])
                attn_h = acts.tile([P, DT, NS], F16, tag="y")
                for h in range(2):
                    qsl = slice(h * 512, (h + 1) * 512)
                    rb = pml.tile([P, 512], F32, tag="pml")
                    nc.tensor.matmul(rb[:], ones_row[:], recip[0:1, qsl])
                    for dt in range(DT):
                        nc.vector.tensor_tensor(
                            attn_h[:, dt, qsl], attacc[:, dt, qsl], rb[:],
                            ALU.mult)
                        nc.vector.tensor_tensor(
                            attn_h[:, dt, qsl], attn_h[:, dt, qsl],
                            bsb["bv"][:, dt:dt + 1].to_broadcast([P, 512]),
                            ALU.add)
                if debug:
                    nc.sync.dma_start(dbg["drs"][:], rs[:])
                    nc.sync.dma_start(
                        dbg["datt"].rearrange("(t p) q -> p t q", p=P),
                        attn_h[:])
                cur = attn_h
                for wname, bname in (("w1", "b1"), ("w2", "b2"), ("w3", "b3")):
                    nxt = acts.tile([P, DT, NS], F16, tag="y")
                    for ft in range(DT):
                        for h in range(2):
                            ps = pml.tile([P, 512], F32, tag="pml")
                            for dt in range(DT):
                                nc.tensor.matmul(
                                    ps[:],
                                    wT[wname][:, dt, ft * P:(ft + 1) * P],
                                    cur[:, dt, h * 512:(h + 1) * 512],
                                    start=(dt == 0), stop=(dt == DT - 1))
                            nc.scalar.activation(
                                nxt[:, ft, h * 512:(h + 1) * 512], ps[:],
                                AF.Relu, bias=bsb[bname][:, ft:ft + 1])
                    if debug and wname == "w1":
                        nc.sync.dma_start(
                            dbg["dy1"].rearrange("(t p) q -> p t q", p=P),
                            nxt[:])
                    cur = nxt
                for h in range(2):
                    ps = pml.tile([1, 512], F32, tag="pfin")
                    for ft in range(DT):
                        nc.tensor.matmul(
                            ps[:], fwh[:, ft:ft + 1],
                            cur[:, ft, h * 512:(h + 1) * 512],
                            start=(ft == 0), stop=(ft == DT - 1))
                    nc.vector.tensor_copy(out_sb[0:1, h * 512:(h + 1) * 512],
                                          ps[:])
                nc.sync.dma_start(out[:], out_sb[:])
            pacc.release()

    nc.compile()
    return nc


def _get_nc():
    if "nc" not in _CACHE:
        _CACHE["nc"] = _build()
    return _CACHE["nc"]


def kernel(**inputs):
    nc = _get_nc()
    x = np.ascontiguousarray(np.asarray(inputs["x"], dtype=np.float32))
    names = {"wq": "Wq", "wk": "Wk", "wv": "Wv", "w1": "W1", "w2": "W2",
             "w3": "W3", "bq": "bq", "bk": "bk", "bv": "bv", "b1": "b1",
             "b2": "b2", "b3": "b3"}
    shared = {k: np.ascontiguousarray(np.asarray(inputs[v], dtype=np.float32))
              for k, v in names.items()}
    shared["fw"] = np.ascontiguousarray(
        np.asarray(inputs["final_weight"], dtype=np.float32).reshape(D))
    in_maps = []
    for c in range(NCORES):
        m = dict(shared)
        m["xs"] = np.ascontiguousarray(x[c * NS:(c + 1) * NS, :])
        in_maps.append(m)
    res = bass_utils.run_bass_kernel_spmd(
        nc, in_maps, core_ids=list(range(NCORES)))
    if os.environ.get("K_DEBUG"):
        kernel.debug_results = res.results
    return np.concatenate(
        [res.results[c]["out"].reshape(NS) for c in range(NCORES)])


# revision 6
# speedup vs baseline: 1.0034x; 1.0034x over previous
"""Trainium2 Bass kernel for DeepSelfAttention (N=8192, D=1024) on 8 NeuronCores.

Strategy (row-parallel attention):
  - Shard the N=8192 rows of x across 8 cores (1024 rows each); replicate weights.
  - All operand transposes (x and the six d x d weights) are done by the DMA
    XBAR (fp32 natural load -> ScalarE fp16 cast -> 8 dma_start_transpose per
    matrix), keeping the TensorEngine free for matmuls.
  - Each core computes Q/K/V projections for its row shard feature-major;
    K^T and V are shipped per key-half: (K^T h0, V h0) -> AllGather0,
    (K^T h1, V h1) -> AllGather1, so the first collective starts as early
    as possible; Q projection + MLP weight transposes fill its latency.
  - Flash-style one-pass attention: scores^T tiles [k=128, q=512] accumulate
    over feature tiles in PSUM, exp on ScalarE (scale=1/32 fused; scores are
    provably in [-3, 3] so no max-subtraction), A@V per (block, dt) with
    free-dim 512 into a rotating set of 4 PSUM banks, flushed to an SBUF
    fp32 accumulator; softmax denominator via a ones-vector matmul.
  - The V bias is folded into the post-softmax normalize (softmax rows sum
    to 1), where it is a per-partition bias.
  - 3-layer MLP + final projection, feature-major.
All matmul operands are fp16 (full PE rate on TRN2) with fp32 PSUM
accumulation; end-to-end max rel err vs the fp32 reference is ~4e-4.
"""

import os

import numpy as np

import concourse.mybir as mybir
import concourse.tile as tile
from concourse import bacc
from concourse import bass_utils

P = 128
D = 1024
N = 8192
NCORES = 8
NS = N // NCORES          # 1024 rows per core
DT = D // P               # 8 feature tiles
KB = 8                    # k blocks (one per source core)
KTB = NS // P             # 8 k tiles per block
KTH = KTB // 2            # 4 k tiles per chunk-block
CH = NS // 2              # 512 keys per chunk
KSZ = D * CH              # K-chunk elements in the flat collective buffer
VSZ = CH * D
F16 = mybir.dt.float16
F32 = mybir.dt.float32
AF = mybir.ActivationFunctionType
ALU = mybir.AluOpType

SCALE = 1.0 / np.sqrt(np.float32(D)).astype(np.float32)  # 0.03125

_CACHE = {}


def _transpose_xbar(nc, st32, st16, src_ap, dst_fn):
    """src_ap: DRAM fp32 [R, C] -> dst_fn(rh, t): SBUF fp16 [P, C//P, P]
    slice receiving columns of src.T for source rows [rh*R/2 + t*P, ...+P),
    via natural load + ScalarE fp16 cast + DMA XBAR transposes, staged in
    row-halves to bound SBUF usage."""
    R, C = src_ap.shape
    tb = R // P // 2
    for rh in range(2):
        nat = st32.tile([P, tb, C], F32, tag="st32")
        nc.sync.dma_start(
            nat[:],
            src_ap[rh * (R // 2):(rh + 1) * (R // 2), :].rearrange(
                "(t p) c -> p t c", p=P))
        nath = st16.tile([P, tb, C], F16, tag="st16")
        nc.scalar.copy(nath[:], nat[:])
        for t in range(tb):
            nc.sync.dma_start_transpose(dst_fn(rh, t), nath[:, t, :])


def _wslice(dst_tile):
    def fn(rh, t):
        i = rh * 4 + t
        return dst_tile[:, :, i * P:(i + 1) * P]
    return fn


def _build():
    nc = bacc.Bacc("TRN2", target_bir_lowering=False, debug=False,
                   num_devices=NCORES)
    xs = nc.dram_tensor("xs", [NS, D], F32, kind="ExternalInput").ap()
    W = {}
    for w in ("wq", "wk", "wv", "w1", "w2", "w3"):
        W[w] = nc.dram_tensor(w, [D, D], F32, kind="ExternalInput").ap()
    B = {}
    for b in ("bq", "bk", "bv", "b1", "b2", "b3"):
        B[b] = nc.dram_tensor(b, [D], F32, kind="ExternalInput").ap()
    fw = nc.dram_tensor("fw", [D], F32, kind="ExternalInput").ap()
    out = nc.dram_tensor("out", [1, NS], F32, kind="ExternalOutput").ap()
    debug = bool(os.environ.get("K_DEBUG"))
    dbg = {}
    if debug:
        for nm, shp, dt_ in (("dq", [D, NS], F16), ("drs", [1, NS], F32),
                             ("datt", [D, NS], F16), ("dy1", [D, NS], F16)):
            dbg[nm] = nc.dram_tensor(nm, shp, dt_, kind="ExternalOutput").ap()

    with tile.TileContext(nc) as tc:
        with (
            tc.tile_pool(name="persist", bufs=1) as pers,
            tc.tile_pool(name="dram", bufs=1, space="DRAM") as dram,
        ):
            # ---- persistent SBUF tiles ----
            qt = pers.tile([P, DT, NS], F16, tag="qt")          # Q^T
            wT = {w: pers.tile([P, DT, D], F16, tag=f"{w}T", name=f"{w}T")
                  for w in ("w1", "w2", "w3")}
            bsb = {b: pers.tile([P, DT], F32, tag=f"{b}sb", name=f"{b}sb")
                   for b in B}
            fwh = pers.tile([P, DT], F16, tag="fwh")
            ones_h = pers.tile([P, 1], F16, tag="ones")
            ones_row = pers.tile([1, P], F32, tag="ones_row")
            rs = pers.tile([1, NS], F32, tag="rs")              # softmax denom

            # ---- DRAM scratch: flat (K-chunk | V-chunk) collective buffers
            kv_d = [dram.tile([KSZ + VSZ], F16, name=f"kv_d{c}")
                    for c in range(2)]
            kvag = [dram.tile([NCORES * (KSZ + VSZ)], F16, name=f"kvag{c}",
                              addr_space="Shared")
                    for c in range(2)]

            # ---- constants ----
            for b in B:
                nc.sync.dma_start(bsb[b][:], B[b].rearrange("(t p) -> p t", p=P))
            fwf = pers.tile([P, DT], F32, tag="fwf")
            nc.sync.dma_start(fwf[:], fw.rearrange("(t p) -> p t", p=P))
            nc.vector.tensor_copy(fwh[:], fwf[:])
            nc.gpsimd.memset(ones_h[:], 1.0)
            nc.gpsimd.memset(ones_row[:], 1.0)

            # ---- early pool: dies after projections ----
            early = tc.alloc_tile_pool(name="early", bufs=1)
            xsT = [early.tile([P, DT, 512], F16, tag=f"xsT{h}",
                              name=f"xsT{h}") for h in range(2)]
            for w in ("wq", "wk", "wv"):
                wT[w] = early.tile([P, DT, D], F16, tag=f"{w}T", name=f"{w}T")
            kts = early.tile([P, DT, NS], F16, tag="kts")       # K^T shard
            vs = early.tile([P, KTB, D], F16, tag="vs")         # V shard

            with (
                tc.tile_pool(name="st32", bufs=2) as st32,
                tc.tile_pool(name="st16", bufs=1) as st16,
                tc.tile_pool(name="ppj", bufs=4, space="PSUM") as ppj,
            ):
                # XBAR transposes: x, then K/V weights (gate AllGather0).
                # x's row-half rh lands in xsT[rh] (rows = keys for K-proj).
                _transpose_xbar(nc, st32, st16, xs,
                                lambda rh, t: xsT[rh][:, :, t * P:(t + 1) * P])
                for w in ("wk", "wv"):
                    _transpose_xbar(nc, st32, st16, W[w], _wslice(wT[w]))

                # per key-half: K^T, V, ship, AllGather
                for h in range(2):
                    # K^T[:, half] = Wk @ xs^T + bk
                    for dt in range(DT):
                        ps = ppj.tile([P, 512], F32, tag="ppj")
                        for et in range(DT):
                            nc.tensor.matmul(
                                ps[:],
                                wT["wk"][:, et, dt * P:(dt + 1) * P],
                                xsT[h][:, et, :],
                                start=(et == 0), stop=(et == DT - 1))
                        nc.scalar.activation(
                            kts[:, dt, h * 512:(h + 1) * 512], ps[:],
                            AF.Identity, bias=bsb["bk"][:, dt:dt + 1])
                    nc.scalar.dma_start(
                        kv_d[h][0:KSZ].rearrange("(t p k) -> p t k", p=P, k=CH),
                        kts[:, :, h * CH:(h + 1) * CH])
                    # V[half] = xs @ Wv.T (bias folded into the normalize)
                    for kt in range(h * KTH, (h + 1) * KTH):
                        for dh in range(2):
                            ps = ppj.tile([P, 512], F32, tag="ppj")
                            for et in range(DT):
                                nc.tensor.matmul(
                                    ps[:],
                                    xsT[h][:, et,
                                           (kt - h * KTH) * P:
                                           (kt - h * KTH + 1) * P],
                                    wT["wv"][:, et, dh * 512:(dh + 1) * 512],
                                    start=(et == 0), stop=(et == DT - 1))
                            nc.scalar.copy(
                                vs[:, kt, dh * 512:(dh + 1) * 512], ps[:])
                    nc.scalar.dma_start(
                        kv_d[h][KSZ:].rearrange("(t p d) -> p t d", p=P, d=D),
                        vs[:, h * KTH:(h + 1) * KTH, :])
                    nc.gpsimd.collective_compute(
                        "AllGather", ALU.bypass,
                        replica_groups=[list(range(NCORES))],
                        ins=[kv_d[h].opt()], outs=[kvag[h].opt()])

                # work that fills the collective latency: Q^T projection
                # + MLP weight transposes
                _transpose_xbar(nc, st32, st16, W["wq"], _wslice(wT["wq"]))
                for dt in range(DT):
                    for h in range(2):
                        ps = ppj.tile([P, 512], F32, tag="ppj")
                        for et in range(DT):
                            nc.tensor.matmul(
                                ps[:],
                                wT["wq"][:, et, dt * P:(dt + 1) * P],
                                xsT[h][:, et, :],
                                start=(et == 0), stop=(et == DT - 1))
                        nc.scalar.activation(
                            qt[:, dt, h * 512:(h + 1) * 512], ps[:],
                            AF.Identity, bias=bsb["bq"][:, dt:dt + 1])
                for w in ("w1", "w2", "w3"):
                    _transpose_xbar(nc, st32, st16, W[w], _wslice(wT[w]))

            early.release()

            if debug:
                nc.sync.dma_start(dbg["dq"].rearrange("(t p) k -> p t k", p=P),
                                  qt[:])

            # ---- attention over 2 chunks x 8 blocks ----
            pacc = tc.alloc_tile_pool(name="pacc", bufs=1)
            attacc = pacc.tile([P, DT, NS], F32, tag="attacc")
            with (
                tc.tile_pool(name="kv", bufs=3) as kv,
                tc.tile_pool(name="ex", bufs=8) as exp_pool,
                tc.tile_pool(name="psc", bufs=2, space="PSUM") as psc,
                tc.tile_pool(name="pat", bufs=4, space="PSUM") as pat,
                tc.tile_pool(name="prs", bufs=2, space="PSUM") as prs,
            ):
                for ch in range(2):
                    base = kvag[ch]
                    for kb in range(KB):
                        off = kb * (KSZ + VSZ)
                        ktb = kv.tile([P, DT, CH], F16, tag="ktb")
                        vb = kv.tile([P, KTH, D], F16, tag="vb")
                        nc.gpsimd.dma_start(
                            ktb[:],
                            base[off:off + KSZ].rearrange(
                                "(t p k) -> p t k", p=P, k=CH))
                        nc.gpsimd.dma_start(
                            vb[:],
                            base[off + KSZ:off + KSZ + VSZ].rearrange(
                                "(t p d) -> p t d", p=P, d=D))
                        first_blk = ch == 0 and kb == 0
                        for qp in range(2):
                            qpsl = slice(qp * 512, (qp + 1) * 512)
                            rs_ps = prs.tile([1, 512], F32, tag="prs")
                            exs = []
                            for kt in range(KTH):
                                sc = psc.tile([P, 512], F32, tag="psc")
                                for dt in range(DT):
                                    nc.tensor.matmul(
                                        sc[:],
                                        ktb[:, dt, kt * P:(kt + 1) * P],
                                        qt[:, dt, qpsl],
                                        start=(dt == 0), stop=(dt == DT - 1))
                                ex = exp_pool.tile([P, 512], F16, tag="ex",
                                                   name=f"ex{kt}")
                                nc.scalar.activation(ex[:], sc[:], AF.Exp,
                                                     scale=float(SCALE))
                                nc.tensor.matmul(rs_ps[:], ones_h[:], ex[:],
                                                 start=(kt == 0),
                                                 stop=(kt == KTH - 1),
                                                 skip_group_check=True)
                                exs.append(ex)
                            if first_blk:
                                nc.vector.tensor_copy(rs[0:1, qpsl], rs_ps[:])
                            else:
                                nc.vector.tensor_tensor(
                                    rs[0:1, qpsl], rs_ps[:], rs[0:1, qpsl],
                                    ALU.add)
                            # A@V: per dt, accumulate the 4 kt matmuls in one
                            # PSUM bank (free dim 512), 4 banks rotating
                            for dt in range(DT):
                                att_ps = pat.tile([P, 512], F32, tag="pat")
                                for kt in range(KTH):
                                    nc.tensor.matmul(
                                        att_ps[:],
                                        vb[:, kt, dt * P:(dt + 1) * P],
                                        exs[kt][:],
                                        start=(kt == 0),
                                        stop=(kt == KTH - 1))
                                dsl = (slice(None), dt, qpsl)
                                if first_blk:
                                    nc.vector.tensor_copy(attacc[dsl],
                                                          att_ps[:])
                                else:
                                    nc.vector.tensor_tensor(
                                        attacc[dsl], att_ps[:],
                                        attacc[dsl], ALU.add)

            # ---- normalize + MLP + final ----
            with (
                tc.tile_pool(name="acts", bufs=2) as acts,
                tc.tile_pool(name="pml", bufs=4, space="PSUM") as pml,
            ):
                recip = acts.tile([1, NS], F32, tag="recip")
                out_sb = acts.tile([1, NS], F32, tag="out_sb")
                nc.vector.reciprocal(recip[:], rs[:])
                attn_h = acts.tile([P, DT, NS], F16, tag="y")
                for h in range(2):
                    qsl = slice(h * 512, (h + 1) * 512)
                    rb = pml.tile([P, 512], F32, tag="pml")
                    nc.tensor.matmul(rb[:], ones_row[:], recip[0:1, qsl])
                    for dt in range(DT):
                        nc.vector.tensor_tensor(
                            attn_h[:, dt, qsl], attacc[:, dt, qsl], rb[:],
                            ALU.mult)
                        nc.vector.tensor_tensor(
                            attn_h[:, dt, qsl], attn_h[:, dt, qsl],
                            bsb["bv"][:, dt:dt + 1].to_broadcast([P, 512]),
                            ALU.add)
                if debug:
                    nc.sync.dma_start(dbg["drs"][:], rs[:])
                    nc.sync.dma_start(
                        dbg["datt"].rearrange("(t p) q -> p t q", p=P),
                        attn_h[:])
                cur = attn_h
                for wname, bname in (("w1", "b1"), ("w2", "b2"), ("w3", "b3")):
                    nxt = acts.tile([P, DT, NS], F16, tag="y")
                    for ft in range(DT):
                        for h in range(2):
                            ps = pml.tile([P, 512], F32, tag="pml")
                            for dt in range(DT):
                                nc.tensor.matmul(
                                    ps[:],
                                    wT[wname][:, dt, ft * P:(ft + 1) * P],
                                    cur[:, dt, h * 512:(h + 1) * 512],
                                    start=(dt == 0), stop=(dt == DT - 1))
                            nc.scalar.activation(
                                nxt[:, ft, h * 512:(h + 1) * 512], ps[:],
                                AF.Relu, bias=bsb[bname][:, ft:ft + 1])
                    if debug and wname == "w1":
                        nc.sync.dma_start(
                            dbg["dy1"].rearrange("(t p) q -> p t q", p=P),
                            nxt[:])
                    cur = nxt
                for h in range(2):
                    ps = pml.tile([1, 512], F32, tag="pfin")
                    for ft in range(DT):
                        nc.tensor.matmul(
                            ps[:], fwh[:, ft:ft + 1],
                            cur[:, ft, h * 512:(h + 1) * 512],
                            start=(ft == 0), stop=(ft == DT - 1))
                    nc.vector.tensor_copy(out_sb[0:1, h * 512:(h + 1) * 512],
                                          ps[:])
                nc.sync.dma_start(out[:], out_sb[:])
            pacc.release()

    nc.compile()
    return nc


def _get_nc():
    if "nc" not in _CACHE:
        _CACHE["nc"] = _build()
    return _CACHE["nc"]


def kernel(**inputs):
    nc = _get_nc()
    x = np.ascontiguousarray(np.asarray(inputs["x"], dtype=np.float32))
    names = {"wq": "Wq", "wk": "Wk", "wv": "Wv", "w1": "W1", "w2": "W2",
             "w3": "W3", "bq": "bq", "bk": "bk", "bv": "bv", "b1": "b1",
             "b2": "b2", "b3": "b3"}
    shared = {k: np.ascontiguousarray(np.asarray(inputs[v], dtype=np.float32))
              for k, v in names.items()}
    shared["fw"] = np.ascontiguousarray(
        np.asarray(inputs["final_weight"], dtype=np.float32).reshape(D))
    in_maps = []
    for c in range(NCORES):
        m = dict(shared)
        m["xs"] = np.ascontiguousarray(x[c * NS:(c + 1) * NS, :])
        in_maps.append(m)
    res = bass_utils.run_bass_kernel_spmd(
        nc, in_maps, core_ids=list(range(NCORES)))
    if os.environ.get("K_DEBUG"):
        kernel.debug_results = res.results
    return np.concatenate(
        [res.results[c]["out"].reshape(NS) for c in range(NCORES)])
